# revision 57
# baseline (speedup 1.0000x reference)
"""Multi-head attention (B=4, L=2048, D=1024, H=16) on 8 Trainium2 NeuronCores.

Sharding: core c = (batch b = c//2, query-half qh = c%2). Each core computes
all 16 heads for its 1024 query rows against the full 2048 keys/values of its
batch. Fully SPMD, no collectives.

All-bf16 data path (fp32 psum). One fully software-pipelined emission
stream: per item (pair, query-half, key-chunk) the S^T matmul pair
(tile_position row-packed, K=64, both heads' scores side by side in one
[128,1024] psum tile) runs two items ahead of the single exp (ACT) and
the PV accumulation, so PE and ACT both stay saturated. Everything else
is interleaved into that stream's slack: the value projection (inside
pair 0's slots, each vpa[j] just before the PV that first needs it),
each next pair's k/q projection chunks (half a contraction chunk per
slot), the ic=0 softmax normalization + first half of the output
projection (inside the final block), with only the ic=1 normalization
and projection half draining at the end. Softmax normalization is
deferred and batched: PV's vpa ones-column yields Z in psum row 64,
Z rows are DMA-gathered into zall, two batched reciprocals + per-pair
selector matmuls broadcast 1/Z, one DVE mul per (pair, ic) normalizes
outU in place. All activations are SBUF-resident end to end (x, w,
outU loaded/kept once; woT reuses wv_sb's storage; no DRAM scratch).
b_v is folded into the output bias host-side (softmax weights sum
to 1). HW exec ~468us vs 1059us for the fp32r phase-serial baseline.
"""

import sys

if "/opt/trn_rl_repo" not in sys.path:
    sys.path.insert(0, "/opt/trn_rl_repo")

import numpy as np

import concourse.bacc as bacc
import concourse.tile as tile
from concourse import mybir
from concourse.bass_utils import run_bass_kernel_spmd

N_CORES = 8
B, L, D = 4, 2048, 1024
NH, DH = 16, 64          # heads, head dim
LQ = L // 2              # query rows per core
F32 = mybir.dt.float32
F32R = mybir.dt.float32r
BF16 = mybir.dt.bfloat16

KC = D // 128            # 8 contraction chunks for projections
NJ = L // 128            # 16 key j-chunks
NI = LQ // 512           # 2 query i-chunks of 512
NPAIR = NH // 2          # 8 head pairs
EXPF = mybir.ActivationFunctionType.Exp


def build_program():
    nc = bacc.Bacc("TRN2", target_bir_lowering=False, debug=False,
                   num_devices=N_CORES)
    with tile.TileContext(nc) as tc:
        _emit(nc, tc)
    nc.compile()
    return nc


def _emit(nc, tc):
    from contextlib import ExitStack

    top = ExitStack()
    dram = top.enter_context(tc.tile_pool(name="dram", bufs=1, space="DRAM"))

    def din(shape, dt, name):
        return dram.tile(shape, dt, kind="ExternalInput", name=name,
                         uniquify=False)

    xqT = din([D, LQ], BF16, "xqT")
    xkT = din([D, L], BF16, "xkT")
    xvT = din([D, L], BF16, "xvT")
    wqT = din([D, D], BF16, "wqT")
    wkT = din([D, D], BF16, "wkT")
    wvT = din([D, D], BF16, "wvT")
    woT = din([D, D], BF16, "woT")
    bqc = din([128, KC], F32, "bqc")
    bkc = din([128, KC], F32, "bkc")
    bor = din([1, D], BF16, "bor")
    c_or = din([1, 128], BF16, "c_or")
    c_sel = din([64, 64 * 4 * NPAIR], F32R, "c_sel")
    out = dram.tile([LQ, D], F32, kind="ExternalOutput", name="out",
                    uniquify=False)

    # ---- persistent SBUF -------------------------------------------------
    pers = top.enter_context(tc.tile_pool(name="pers", bufs=1))
    kpT = [pers.tile([128, L], BF16, name=f"kpT{m}") for m in range(NPAIR)]
    qpT = [pers.tile([128, LQ], BF16, name=f"qpT{m}") for m in range(NPAIR)]
    # vpa: per j-chunk, 16 heads x (64 value cols + 1 ones col + 1 pad col
    # so every head's 65-col weight slice starts 4B-aligned)
    VST = 66
    vpa = [pers.tile([128, NH * VST], BF16, name=f"vpa{m}") for m in range(NJ)]
    outU = [pers.tile([128, LQ], BF16, name=f"outU{m}") for m in range(NPAIR)]
    xk_sb = pers.tile([128, KC, L], BF16, name="xk_sb")
    xq_sb = pers.tile([128, KC, LQ], BF16, name="xq_sb")
    wv_sb = pers.tile([128, KC, D], BF16, name="wv_sb")
    ones1 = pers.tile([1, 128], BF16, name="ones1")
    # b_v's contribution to the output is bv @ woT (softmax weights sum to
    # 1), folded into bor host-side; no bias term needed in vp itself.
    # 64 partitions (rows 32+ pad: zall 1.0, sel 0) for valid 64x64 PE tiling
    sel_sb = pers.tile([64, 64 * 4 * NPAIR], F32R, name="sel_sb")
    zall = pers.tile([64, 512], F32, name="zall")
    bq_sb = pers.tile([128, KC], F32, name="bq_sb")
    bk_sb = pers.tile([128, KC], F32, name="bk_sb")
    bo_sb = pers.tile([1, D], BF16, name="bo_sb")

    xkT_r = xkT.rearrange("(kc p) l -> p kc l", p=128)
    xqT_r = xqT.rearrange("(kc p) l -> p kc l", p=128)
    xvT_r = xvT.rearrange("(kc p) l -> p kc l", p=128)
    wqT_r = wqT.rearrange("(kc p) m -> p kc m", p=128)
    wkT_r = wkT.rearrange("(kc p) m -> p kc m", p=128)
    wvT_r = wvT.rearrange("(kc p) m -> p kc m", p=128)
    woT_r = woT.rearrange("(kc p) m -> p kc m", p=128)

    # ---- phases 1 + 2: projections interleaved with attention -----------
    # PSUM budget (8 banks): psAB 2 bufs x [128,1024] = 4, psO 2 tags x 1
    # buf x [65,512] = 2, ppk (kq chunks AND vp chunks) 2 x [128,512] = 2.
    attn_ctx = ExitStack()
    pw = attn_ctx.enter_context(tc.tile_pool(name="pw", bufs=2))
    pxv = attn_ctx.enter_context(tc.tile_pool(name="pxv", bufs=3))
    ppk = attn_ctx.enter_context(tc.tile_pool(name="ppk", bufs=2, space="PSUM"))
    psA = attn_ctx.enter_context(tc.tile_pool(name="psA", bufs=2, space="PSUM"))
    psO = attn_ctx.enter_context(tc.tile_pool(name="psO", bufs=1, space="PSUM"))
    pe = attn_ctx.enter_context(tc.tile_pool(name="pe", bufs=3))
    prc = attn_ctx.enter_context(tc.tile_pool(name="prc", bufs=2))

    xv_pre = {}

    def stage_xv(m):
        xb = pxv.tile([128, KC, 128], BF16, tag="xv", name=f"xv{m}")
        nc.sync.dma_start(out=xb[:], in_=xvT_r[:, :, m * 128:(m + 1) * 128])
        xv_pre[m] = xb

    def emit_vp(m):
        # value projection for key chunk m -> vpa[m] (emitted just before
        # the PV that first consumes it, inside pair 0's stream)
        if m not in xv_pre:
            stage_xv(m)
        xb = xv_pre.pop(m)
        va = vpa[m].rearrange("p (h c) -> p h c", c=VST)
        nc.vector.memset(va[:, :, 64:65], 1.0)
        for n in range(2):
            nsl = slice(n * 512, (n + 1) * 512)
            ps = ppk.tile([128, 512], F32, tag="pk", name=f"pv{m}_{n}")
            for kc in range(KC):
                nc.tensor.matmul(ps[:], xb[:, kc, :], wv_sb[:, kc, nsl],
                                 start=(kc == 0), stop=(kc == KC - 1))
            nc.vector.tensor_copy(va[:, 8 * n:8 * (n + 1), 0:64], ps[:])

    wk_tiles = {}

    def emit_kq_load(p):
        wkb = pw.tile([128, KC, 128], BF16, tag="wk")
        nc.sync.dma_start(out=wkb[:], in_=wkT_r[:, :, p * 128:(p + 1) * 128])
        wqb = pw.tile([128, KC, 128], BF16, tag="wq")
        nc.sync.dma_start(out=wqb[:], in_=wqT_r[:, :, p * 128:(p + 1) * 128])
        wk_tiles[p] = (wkb, wqb)

    kq_ps = {}

    def emit_kq_chunk(p, c, half=None):
        # chunks 0-3: kpT[p] 512-col chunk c; chunks 4-5: qpT[p] chunk c-4
        # half=0/1 emits only the lower/upper kc contraction half (so the
        # matmul burst can be split across two pipeline slots)
        wkb, wqb = wk_tiles[p]
        if c < 4:
            wb, x_sb, dst, bias, cc = wkb, xk_sb, kpT[p], bk_sb, c
        else:
            wb, x_sb, dst, bias, cc = wqb, xq_sb, qpT[p], bq_sb, c - 4
        csl = slice(cc * 512, (cc + 1) * 512)
        if half in (None, 0):
            ps = ppk.tile([128, 512], F32, tag="pk", name=f"pk{p}_{c}")
            kq_ps[(p, c)] = ps
        else:
            ps = kq_ps.pop((p, c))
        kcs = range(KC) if half is None else range(half * KC // 2,
                                                  (half + 1) * KC // 2)
        for kc in kcs:
            nc.tensor.matmul(ps[:], wb[:, kc, :], x_sb[:, kc, csl],
                             start=(kc == 0), stop=(kc == KC - 1))
        if half in (None, 1):
            nc.vector.tensor_scalar_add(dst[:, csl], ps[:], bias[:, p:p + 1])

    # pair-0 prefix: just the chunks the first S items need (kpT cols 0:512,
    # qpT ic=0); the rest streams inside pair 0's item slots. DMAs are
    # emitted in critical-path order: pair-0 weights + first x slices first
    # so the PE (and then ACT) starts within a few us.
    emit_kq_load(0)
    nc.sync.dma_start(out=xk_sb[:, :, 0:512], in_=xkT_r[:, :, 0:512])
    nc.sync.dma_start(out=xq_sb[:, :, 0:512], in_=xqT_r[:, :, 0:512])
    nc.sync.dma_start(out=bq_sb[:], in_=bqc[:])
    nc.sync.dma_start(out=bk_sb[:], in_=bkc[:])
    emit_kq_chunk(0, 0)
    emit_kq_chunk(0, 4)
    nc.sync.dma_start(out=wv_sb[:], in_=wvT_r[:])
    for m_ in range(3):
        stage_xv(m_)
    nc.sync.dma_start(out=ones1[:], in_=c_or[:])
    # bulk loads ride the ACT engine's DGE ring so the sync ring keeps
    # feeding the vp stream without queueing behind them
    for c in range(1, 4):
        nc.scalar.dma_start(out=xk_sb[:, :, c * 512:(c + 1) * 512],
                            in_=xkT_r[:, :, c * 512:(c + 1) * 512])
    nc.scalar.dma_start(out=xq_sb[:, :, 512:1024], in_=xqT_r[:, :, 512:1024])
    nc.scalar.dma_start(out=sel_sb[:], in_=c_sel[:])
    nc.scalar.dma_start(out=bo_sb[:], in_=bor[:])

    # flattened item stream: one item = one key j-chunk of one (pair, ic)
    # block. Scores for both heads of the pair sit side by side in one
    # [128, 1024] psum tile so a single exp op covers them.
    items = [(p, ic, j)
             for p in range(NPAIR) for ic in range(NI) for j in range(NJ)]
    s_t, e_t = {}, {}
    oz = {}

    def emit_S(k):
        p, ic, j = items[k]
        isl = slice(ic * 512, (ic + 1) * 512)
        jsl = slice(j * 128, (j + 1) * 128)
        s = psA.tile([128, 1024], F32, tag="s")
        nc.tensor.matmul(s[:, 0:512], kpT[p][0:64, jsl], qpT[p][0:64, isl],
                         tile_position=(0, 0))
        nc.tensor.matmul(s[:, 512:1024], kpT[p][64:128, jsl],
                         qpT[p][64:128, isl], tile_position=(64, 0))
        s_t[k] = s

    def emit_exp(k):
        s = s_t.pop(k)
        e = pe.tile([128, 1024], BF16, tag="e")
        nc.scalar.activation(e[:], s[:], EXPF)
        e_t[k] = e

    def emit_PV(k):
        p, ic, j = items[k]
        hA, hB = 2 * p, 2 * p + 1
        if j == 0:
            ozA = psO.tile([65, 512], F32, tag="oa", name=f"ozA{p}_{ic}")
            ozB = psO.tile([65, 512], F32, tag="ob", name=f"ozB{p}_{ic}")
            oz[(p, ic)] = (ozA, ozB)
        ozA, ozB = oz[(p, ic)]
        e = e_t.pop(k)
        # B half first: the second PV after an S pair consistently ran
        # ~100ns slower; give the B matmul the better prefetch slot
        nc.tensor.matmul(ozB[:, :], vpa[j][:, hB * VST:hB * VST + 65],
                         e[:, 512:1024], start=(j == 0), stop=(j == NJ - 1))
        nc.tensor.matmul(ozA[:, :], vpa[j][:, hA * VST:hA * VST + 65],
                         e[:, 0:512], start=(j == 0), stop=(j == NJ - 1))

    def emit_block_end(k):
        p, ic, j = items[k]
        ozA, ozB = oz.pop((p, ic))
        isl = slice(ic * 512, (ic + 1) * 512)
        # unnormalized head outputs -> SBUF (normalized in place later)
        nc.vector.tensor_copy(outU[p][0:64, isl], ozA[0:64, :])
        nc.vector.tensor_copy(outU[p][64:128, isl], ozB[0:64, :])
        # Z rows (psum partition 64) -> zall rows via staging + DMA
        zst = prc.tile([65, 1024], F32, tag="zs")
        nc.vector.tensor_copy(zst[64:65, 0:512], ozA[64:65, :])
        nc.vector.tensor_copy(zst[64:65, 512:1024], ozB[64:65, :])
        r0 = 4 * p + 2 * ic
        nc.sync.dma_start(out=zall[r0:r0 + 1, :], in_=zst[64:65, 0:512])
        nc.sync.dma_start(out=zall[r0 + 1:r0 + 2, :], in_=zst[64:65, 512:1024])

    # ---- deferred units: per-block normalization + output projection ----
    # Each block's softmax normalization streams into the item loop right
    # after the block drains (a full-table reciprocal is cheap and
    # idempotent on already-final rows); the ic=0 half of the output
    # projection streams into the last block. Only pair 7's ic=1 norm and
    # the ic=1 projection half remain after the loop.
    fs = attn_ctx.enter_context(tc.tile_pool(name="fs", bufs=2))
    pn = attn_ctx.enter_context(tc.tile_pool(name="pn", bufs=1))
    rz = pn.tile([64, 512], F32R, name="rz")

    def emit_recip():
        # one batched reciprocal per ic phase — the DVE reciprocal is an
        # iterative ~4us op, so per-block recips stall the PE badly
        with nc.allow_low_precision(reason="fp32r rounding of 1/Z"):
            nc.vector.reciprocal(rz[:], zall[:])

    def emit_norm(p, ic):
        r0 = 4 * p + 2 * ic
        isl = slice(ic * 512, (ic + 1) * 512)
        rzb = ppk.tile([128, 512], F32, tag="pk", name=f"rzb{p}_{ic}")
        nc.tensor.matmul(rzb[:, :], sel_sb[:, r0 * 64:(r0 + 2) * 64], rz[:])
        nc.vector.tensor_mul(outU[p][:, isl], outU[p][:, isl], rzb[:])

    def emit_ph3(n, m):
        nsl = slice(n * 512, (n + 1) * 512)
        msl = slice(m * 128, (m + 1) * 128)
        ps = ppk.tile([128, 512], F32, tag="pk", name=f"pf{n}_{m}")
        nc.tensor.matmul(ps[:], ones1[0:1, :], bo_sb[0:1, nsl],
                         start=True, stop=False)
        for kc in range(KC):
            nc.tensor.matmul(ps[:], outU[kc][:, msl], wv_sb[:, kc, nsl],
                             start=False, stop=(kc == KC - 1))
        ost = fs.tile([128, 512], F32, tag="fs", name=f"fo{n}_{m}")
        # alternate copy engines AND DMA rings (SP + ACT both have
        # hardware DGE queues) so nothing single-threads the tail
        if m % 2 == 0:
            nc.vector.tensor_copy(ost[:], ps[:])
            nc.sync.dma_start(out=out[msl, nsl], in_=ost[:])
        else:
            nc.scalar.copy(ost[:], ps[:])
            nc.scalar.dma_start(out=out[msl, nsl], in_=ost[:])

    defer_q = []

    # software-pipelined emission: S runs 2 items ahead of exp/PV
    NIT = len(items)
    emit_S(0)
    emit_exp(0)
    emit_S(1)
    emit_exp(1)
    nc.vector.memset(zall[:, :], 1.0)
    # remaining pair-0 chunks stream inside pair 0 itself; for pairs 1+ the
    # qpT/kpT chunks a pair's first S items touch are emitted a pair early
    # (q chunks first so the lookahead S of the next pair never outruns its
    # DVE producer in queue order)
    # pair-0 chunks stay unsplit: they interleave with vp chunks in the
    # shared ppk pool, and a half-open accumulation there would deadlock
    # the pool's WAR rotation
    kq_work = [(0, c, None) for c in (1, 2, 3, 5)]
    for p in range(1, NPAIR):
        kq_work.append((p, -1, None))
        for c in (4, 5, 0, 1, 2, 3):
            kq_work.append((p, c, 0))
            kq_work.append((p, c, 1))
    kq_i = 0
    for k in range(NIT):
        p_cur, ic_cur, j_cur = items[k]
        if p_cur == 0 and ic_cur == 0:
            emit_vp(j_cur)
        emit_PV(k)
        if k == NIT - 1 or items[k + 1][2] == 0:
            emit_block_end(k)
            if (p_cur, ic_cur) == (0, 1):
                # vp is done; woT reuses wv_sb's storage for phase 3
                defer_q.append(
                    lambda: nc.sync.dma_start(out=wv_sb[:], in_=woT_r[:]))
            if (p_cur, ic_cur) == (NPAIR - 1, 0):
                # ic=0 normalization + projection half streams into the
                # final block (its inputs are final once (p7, ic0) drains);
                # the reciprocal goes out right away so the DVE computes it
                # under the block's last PVs instead of stalling the stream
                emit_recip()
                for p_ in range(NPAIR):
                    defer_q.append(lambda p=p_: emit_norm(p, 0))
                for n_ in range(2):
                    for m_ in range(4):
                        defer_q.append(lambda n=n_, m=m_: emit_ph3(n, m))
        # interleave upcoming projection work into this pair's stream,
        # half a contraction chunk per slot to keep the PE queue smooth
        slot = ic_cur * NJ + j_cur
        if slot % 2 == 0:
            while kq_i < len(kq_work) and kq_work[kq_i][0] <= p_cur + 1:
                pp_, cc_, hh_ = kq_work[kq_i]
                if cc_ < 0:
                    emit_kq_load(pp_)
                else:
                    emit_kq_chunk(pp_, cc_, hh_)
                kq_i += 1
                if cc_ >= 0:
                    break
        # deferred norm/projection units: one per slot, only while no kq
        # psum accumulation is half-open in the shared ppk pool
        if defer_q and not kq_ps:
            defer_q.pop(0)()
        if k + 2 < NIT:
            emit_S(k + 2)
            emit_exp(k + 2)
    # drain: ic=1 normalization + remaining output projection
    for fn in defer_q:
        fn()
    emit_recip()
    for p in range(NPAIR):
        emit_norm(p, 1)
    for n in range(2):
        for m in range(4, LQ // 128):
            emit_ph3(n, m)
    attn_ctx.close()


_NC_CACHE = None


def _get_program():
    global _NC_CACHE
    if _NC_CACHE is None:
        _NC_CACHE = build_program()
    return _NC_CACHE


def prep_in_maps(q, k, v, w_q, b_q, w_k, b_k, w_v, b_v, w_o, b_o):
    import ml_dtypes

    f = np.float32
    bf = ml_dtypes.bfloat16
    q, k, v = (np.asarray(t, f) for t in (q, k, v))
    scale = 1.0 / np.sqrt(DH)
    wqT = np.ascontiguousarray((np.asarray(w_q, f) * scale).T).astype(bf)
    wkT = np.ascontiguousarray(np.asarray(w_k, f).T).astype(bf)
    wvT = np.ascontiguousarray(np.asarray(w_v, f).T).astype(bf)
    woT = np.ascontiguousarray(np.asarray(w_o, f).T).astype(bf)
    bqc = np.ascontiguousarray((np.asarray(b_q, f) * scale).reshape(KC, 128).T)
    bkc = np.ascontiguousarray(np.asarray(b_k, f).reshape(KC, 128).T)
    # softmax weights sum to 1, so b_v contributes exactly b_v @ w_o.T to
    # the output; fold it into the output-projection bias
    bor = ((np.asarray(b_v, f) @ np.asarray(w_o, f).T)
           + np.asarray(b_o, f)).reshape(1, D).astype(bf)
    c_or = np.ones((1, 128), bf)
    c_sel = np.zeros((64, 64 * 4 * NPAIR), f)
    for r in range(4 * NPAIR):
        c_sel[r, r * 64:(r + 1) * 64] = 1.0
    in_maps = []
    for c in range(N_CORES):
        b, qh = c // 2, c % 2
        kTb = np.ascontiguousarray(k[b].T).astype(bf)
        vTb = np.ascontiguousarray(v[b].T).astype(bf)
        qTb = np.ascontiguousarray(q[b].T[:, qh * LQ:(qh + 1) * LQ]).astype(bf)
        in_maps.append({
            "xqT": qTb, "xkT": kTb, "xvT": vTb,
            "wqT": wqT, "wkT": wkT, "wvT": wvT, "woT": woT,
            "bqc": bqc, "bkc": bkc, "bor": bor,
            "c_or": c_or, "c_sel": c_sel,
        })
    return in_maps


def run(in_maps, trace=False, **kw):
    nc = _get_program()
    return run_bass_kernel_spmd(nc, in_maps, list(range(N_CORES)),
                                trace=trace, **kw)


def kernel(**inputs):
    in_maps = prep_in_maps(**inputs)
    res = run(in_maps)
    out = np.empty((B, L, D), np.float32)
    for c in range(N_CORES):
        b, qh = c // 2, c % 2
        out[b, qh * LQ:(qh + 1) * LQ, :] = res.results[c]["out"]
    return out


# revision 61
# speedup vs baseline: 1.0214x; 1.0214x over previous
"""Multi-head attention (B=4, L=2048, D=1024, H=16) on 8 Trainium2 NeuronCores.

Sharding: core c = (batch b = c//2, query-half qh = c%2). Each core computes
all 16 heads for its 1024 query rows against the full 2048 keys/values of its
batch. Fully SPMD, no collectives.

All-bf16 data path (fp32 psum). One fully software-pipelined emission
stream: per item (pair, query-half, key-chunk) the S^T matmul pair
(tile_position row-packed, K=64, both heads' scores side by side in one
[128,1024] psum tile) runs two items ahead of the single exp (ACT) and
the PV accumulation, so PE and ACT both stay saturated. Everything else
is interleaved into that stream's slack: the value projection (inside
pair 0's slots, each vpa[j] just before the PV that first needs it),
each next pair's k/q projection chunks (half a contraction chunk per
slot), the ic=0 softmax normalization + first half of the output
projection (inside the final block), with only the ic=1 normalization
and projection half draining at the end. Softmax normalization is
deferred and batched: PV's vpa ones-column yields Z in psum row 64,
Z rows are DMA-gathered into zall, two batched reciprocals + per-pair
selector matmuls broadcast 1/Z, one DVE mul per (pair, ic) normalizes
outU in place. All activations are SBUF-resident end to end (x, w,
outU loaded/kept once; woT reuses wv_sb's storage; no DRAM scratch).
b_v is folded into the output bias host-side (softmax weights sum
to 1). HW exec ~468us vs 1059us for the fp32r phase-serial baseline.
"""

import sys

if "/opt/trn_rl_repo" not in sys.path:
    sys.path.insert(0, "/opt/trn_rl_repo")

import numpy as np

import concourse.bacc as bacc
import concourse.tile as tile
from concourse import mybir
from concourse.bass_utils import run_bass_kernel_spmd

N_CORES = 8
B, L, D = 4, 2048, 1024
NH, DH = 16, 64          # heads, head dim
LQ = L // 2              # query rows per core
F32 = mybir.dt.float32
F32R = mybir.dt.float32r
BF16 = mybir.dt.bfloat16

KC = D // 128            # 8 contraction chunks for projections
NJ = L // 128            # 16 key j-chunks
NI = LQ // 512           # 2 query i-chunks of 512
NPAIR = NH // 2          # 8 head pairs
EXPF = mybir.ActivationFunctionType.Exp


def build_program():
    nc = bacc.Bacc("TRN2", target_bir_lowering=False, debug=False,
                   num_devices=N_CORES)
    with tile.TileContext(nc) as tc:
        _emit(nc, tc)
    nc.compile()
    return nc


def _emit(nc, tc):
    from contextlib import ExitStack

    top = ExitStack()
    dram = top.enter_context(tc.tile_pool(name="dram", bufs=1, space="DRAM"))

    def din(shape, dt, name):
        return dram.tile(shape, dt, kind="ExternalInput", name=name,
                         uniquify=False)

    xqT = din([D, LQ], BF16, "xqT")
    xkT = din([D, L], BF16, "xkT")
    xvT = din([D, L], BF16, "xvT")
    wqT = din([D, D], BF16, "wqT")
    wkT = din([D, D], BF16, "wkT")
    wvT = din([D, D], BF16, "wvT")
    woT = din([D, D], BF16, "woT")
    bqc = din([128, KC], F32, "bqc")
    bkc = din([128, KC], F32, "bkc")
    bor = din([1, D], BF16, "bor")
    c_or = din([1, 128], BF16, "c_or")
    c_sel = din([64, 64 * 4 * NPAIR], F32R, "c_sel")
    out = dram.tile([LQ, D], F32, kind="ExternalOutput", name="out",
                    uniquify=False)

    # ---- persistent SBUF -------------------------------------------------
    pers = top.enter_context(tc.tile_pool(name="pers", bufs=1))
    kpT = [pers.tile([128, L], BF16, name=f"kpT{m}") for m in range(NPAIR)]
    qpT = [pers.tile([128, LQ], BF16, name=f"qpT{m}") for m in range(NPAIR)]
    # vpa: per j-chunk, 16 heads x (64 value cols + 1 ones col + 1 pad col
    # so every head's 65-col weight slice starts 4B-aligned)
    VST = 66
    vpa = [pers.tile([128, NH * VST], BF16, name=f"vpa{m}") for m in range(NJ)]
    outU = [pers.tile([128, LQ], BF16, name=f"outU{m}") for m in range(NPAIR)]
    xk_sb = pers.tile([128, KC, L], BF16, name="xk_sb")
    xq_sb = pers.tile([128, KC, LQ], BF16, name="xq_sb")
    wv_sb = pers.tile([128, KC, D], BF16, name="wv_sb")
    ones1 = pers.tile([1, 128], BF16, name="ones1")
    # b_v's contribution to the output is bv @ woT (softmax weights sum to
    # 1), folded into bor host-side; no bias term needed in vp itself.
    # 64 partitions (rows 32+ pad: zall 1.0, sel 0) for valid 64x64 PE tiling
    sel_sb = pers.tile([64, 64 * 4 * NPAIR], F32R, name="sel_sb")
    zall = pers.tile([64, 512], F32, name="zall")
    bq_sb = pers.tile([128, KC], F32, name="bq_sb")
    bk_sb = pers.tile([128, KC], F32, name="bk_sb")
    bo_sb = pers.tile([1, D], BF16, name="bo_sb")

    xkT_r = xkT.rearrange("(kc p) l -> p kc l", p=128)
    xqT_r = xqT.rearrange("(kc p) l -> p kc l", p=128)
    xvT_r = xvT.rearrange("(kc p) l -> p kc l", p=128)
    wqT_r = wqT.rearrange("(kc p) m -> p kc m", p=128)
    wkT_r = wkT.rearrange("(kc p) m -> p kc m", p=128)
    wvT_r = wvT.rearrange("(kc p) m -> p kc m", p=128)
    woT_r = woT.rearrange("(kc p) m -> p kc m", p=128)

    # ---- phases 1 + 2: projections interleaved with attention -----------
    # PSUM budget (8 banks): psAB 2 bufs x [128,1024] = 4, psO 2 tags x 1
    # buf x [65,512] = 2, ppk (kq chunks AND vp chunks) 2 x [128,512] = 2.
    attn_ctx = ExitStack()
    pw = attn_ctx.enter_context(tc.tile_pool(name="pw", bufs=2))
    pxv = attn_ctx.enter_context(tc.tile_pool(name="pxv", bufs=3))
    ppk = attn_ctx.enter_context(tc.tile_pool(name="ppk", bufs=2, space="PSUM"))
    psA = attn_ctx.enter_context(tc.tile_pool(name="psA", bufs=2, space="PSUM"))
    psO = attn_ctx.enter_context(tc.tile_pool(name="psO", bufs=1, space="PSUM"))
    pe = attn_ctx.enter_context(tc.tile_pool(name="pe", bufs=3))
    prc = attn_ctx.enter_context(tc.tile_pool(name="prc", bufs=2))

    xv_pre = {}

    def stage_xv(m):
        xb = pxv.tile([128, KC, 128], BF16, tag="xv", name=f"xv{m}")
        nc.sync.dma_start(out=xb[:], in_=xvT_r[:, :, m * 128:(m + 1) * 128])
        xv_pre[m] = xb

    def emit_vp(m):
        # value projection for key chunk m -> vpa[m] (emitted just before
        # the PV that first consumes it, inside pair 0's stream)
        if m not in xv_pre:
            stage_xv(m)
        xb = xv_pre.pop(m)
        va = vpa[m].rearrange("p (h c) -> p h c", c=VST)
        nc.vector.memset(va[:, :, 64:65], 1.0)
        for n in range(2):
            nsl = slice(n * 512, (n + 1) * 512)
            ps = ppk.tile([128, 512], F32, tag="pk", name=f"pv{m}_{n}")
            for kc in range(KC):
                nc.tensor.matmul(ps[:], xb[:, kc, :], wv_sb[:, kc, nsl],
                                 start=(kc == 0), stop=(kc == KC - 1))
            nc.vector.tensor_copy(va[:, 8 * n:8 * (n + 1), 0:64], ps[:])

    wk_tiles = {}

    def emit_kq_load(p):
        wkb = pw.tile([128, KC, 128], BF16, tag="wk")
        nc.sync.dma_start(out=wkb[:], in_=wkT_r[:, :, p * 128:(p + 1) * 128])
        wqb = pw.tile([128, KC, 128], BF16, tag="wq")
        nc.sync.dma_start(out=wqb[:], in_=wqT_r[:, :, p * 128:(p + 1) * 128])
        wk_tiles[p] = (wkb, wqb)

    kq_ps = {}

    def emit_kq_chunk(p, c, half=None):
        # chunks 0-3: kpT[p] 512-col chunk c; chunks 4-5: qpT[p] chunk c-4
        # half=0/1 emits only the lower/upper kc contraction half (so the
        # matmul burst can be split across two pipeline slots)
        wkb, wqb = wk_tiles[p]
        if c < 4:
            wb, x_sb, dst, bias, cc = wkb, xk_sb, kpT[p], bk_sb, c
        else:
            wb, x_sb, dst, bias, cc = wqb, xq_sb, qpT[p], bq_sb, c - 4
        csl = slice(cc * 512, (cc + 1) * 512)
        if half in (None, 0):
            ps = ppk.tile([128, 512], F32, tag="pk", name=f"pk{p}_{c}")
            kq_ps[(p, c)] = ps
        else:
            ps = kq_ps.pop((p, c))
        kcs = range(KC) if half is None else range(half * KC // 2,
                                                  (half + 1) * KC // 2)
        for kc in kcs:
            nc.tensor.matmul(ps[:], wb[:, kc, :], x_sb[:, kc, csl],
                             start=(kc == 0), stop=(kc == KC - 1))
        if half in (None, 1):
            nc.vector.tensor_scalar_add(dst[:, csl], ps[:], bias[:, p:p + 1])

    # pair-0 prefix: just the chunks the first S items need (kpT cols 0:512,
    # qpT ic=0); the rest streams inside pair 0's item slots. DMAs are
    # emitted in critical-path order: pair-0 weights + first x slices first
    # so the PE (and then ACT) starts within a few us.
    emit_kq_load(0)
    nc.sync.dma_start(out=xk_sb[:, :, 0:512], in_=xkT_r[:, :, 0:512])
    nc.sync.dma_start(out=xq_sb[:, :, 0:512], in_=xqT_r[:, :, 0:512])
    nc.sync.dma_start(out=bq_sb[:], in_=bqc[:])
    nc.sync.dma_start(out=bk_sb[:], in_=bkc[:])
    emit_kq_chunk(0, 0)
    emit_kq_chunk(0, 4)
    nc.sync.dma_start(out=wv_sb[:], in_=wvT_r[:])
    for m_ in range(3):
        stage_xv(m_)
    nc.sync.dma_start(out=ones1[:], in_=c_or[:])
    for c in range(1, 4):
        nc.sync.dma_start(out=xk_sb[:, :, c * 512:(c + 1) * 512],
                          in_=xkT_r[:, :, c * 512:(c + 1) * 512])
    nc.sync.dma_start(out=xq_sb[:, :, 512:1024], in_=xqT_r[:, :, 512:1024])
    nc.sync.dma_start(out=sel_sb[:], in_=c_sel[:])
    nc.sync.dma_start(out=bo_sb[:], in_=bor[:])

    # flattened item stream: one item = one key j-chunk of one (pair, ic)
    # block. Scores for both heads of the pair sit side by side in one
    # [128, 1024] psum tile so a single exp op covers them.
    items = [(p, ic, j)
             for p in range(NPAIR) for ic in range(NI) for j in range(NJ)]
    s_t, e_t = {}, {}
    oz = {}

    def emit_S(k):
        p, ic, j = items[k]
        isl = slice(ic * 512, (ic + 1) * 512)
        jsl = slice(j * 128, (j + 1) * 128)
        s = psA.tile([128, 1024], F32, tag="s")
        nc.tensor.matmul(s[:, 0:512], kpT[p][0:64, jsl], qpT[p][0:64, isl],
                         tile_position=(0, 0))
        nc.tensor.matmul(s[:, 512:1024], kpT[p][64:128, jsl],
                         qpT[p][64:128, isl], tile_position=(64, 0))
        s_t[k] = s

    def emit_exp(k):
        s = s_t.pop(k)
        e = pe.tile([128, 1024], BF16, tag="e")
        nc.scalar.activation(e[:], s[:], EXPF)
        e_t[k] = e

    def emit_PV_B(k):
        p, ic, j = items[k]
        hB = 2 * p + 1
        if j == 0:
            ozA = psO.tile([65, 512], F32, tag="oa", name=f"ozA{p}_{ic}")
            ozB = psO.tile([65, 512], F32, tag="ob", name=f"ozB{p}_{ic}")
            oz[(p, ic)] = (ozA, ozB)
        ozA, ozB = oz[(p, ic)]
        e = e_t[k]
        nc.tensor.matmul(ozB[:, :], vpa[j][:, hB * VST:hB * VST + 65],
                         e[:, 512:1024], start=(j == 0), stop=(j == NJ - 1))

    def emit_PV_A(k):
        # emitted a few PE slots after PV_B: back-to-back PVs consistently
        # cost the second one ~100ns, so each PV gets its own post-S slot
        p, ic, j = items[k]
        hA = 2 * p
        ozA, ozB = oz[(p, ic)]
        e = e_t.pop(k)
        nc.tensor.matmul(ozA[:, :], vpa[j][:, hA * VST:hA * VST + 65],
                         e[:, 0:512], start=(j == 0), stop=(j == NJ - 1))

    def emit_block_end(k):
        p, ic, j = items[k]
        ozA, ozB = oz.pop((p, ic))
        isl = slice(ic * 512, (ic + 1) * 512)
        # unnormalized head outputs -> SBUF (normalized in place later)
        nc.vector.tensor_copy(outU[p][0:64, isl], ozA[0:64, :])
        nc.vector.tensor_copy(outU[p][64:128, isl], ozB[0:64, :])
        # Z rows (psum partition 64) -> zall rows via staging + DMA
        zst = prc.tile([65, 1024], F32, tag="zs")
        nc.vector.tensor_copy(zst[64:65, 0:512], ozA[64:65, :])
        nc.vector.tensor_copy(zst[64:65, 512:1024], ozB[64:65, :])
        r0 = 4 * p + 2 * ic
        nc.sync.dma_start(out=zall[r0:r0 + 1, :], in_=zst[64:65, 0:512])
        nc.sync.dma_start(out=zall[r0 + 1:r0 + 2, :], in_=zst[64:65, 512:1024])

    # ---- deferred units: per-block normalization + output projection ----
    # Each block's softmax normalization streams into the item loop right
    # after the block drains (a full-table reciprocal is cheap and
    # idempotent on already-final rows); the ic=0 half of the output
    # projection streams into the last block. Only pair 7's ic=1 norm and
    # the ic=1 projection half remain after the loop.
    fs = attn_ctx.enter_context(tc.tile_pool(name="fs", bufs=2))
    pn = attn_ctx.enter_context(tc.tile_pool(name="pn", bufs=1))
    rz = pn.tile([64, 512], F32R, name="rz")

    def emit_recip():
        # one batched reciprocal per ic phase — the DVE reciprocal is an
        # iterative ~4us op, so per-block recips stall the PE badly
        with nc.allow_low_precision(reason="fp32r rounding of 1/Z"):
            nc.vector.reciprocal(rz[:], zall[:])

    def emit_norm(p, ic):
        r0 = 4 * p + 2 * ic
        isl = slice(ic * 512, (ic + 1) * 512)
        rzb = ppk.tile([128, 512], F32, tag="pk", name=f"rzb{p}_{ic}")
        nc.tensor.matmul(rzb[:, :], sel_sb[:, r0 * 64:(r0 + 2) * 64], rz[:])
        nc.vector.tensor_mul(outU[p][:, isl], outU[p][:, isl], rzb[:])

    def emit_ph3(n, m):
        nsl = slice(n * 512, (n + 1) * 512)
        msl = slice(m * 128, (m + 1) * 128)
        ps = ppk.tile([128, 512], F32, tag="pk", name=f"pf{n}_{m}")
        nc.tensor.matmul(ps[:], ones1[0:1, :], bo_sb[0:1, nsl],
                         start=True, stop=False)
        for kc in range(KC):
            nc.tensor.matmul(ps[:], outU[kc][:, msl], wv_sb[:, kc, nsl],
                             start=False, stop=(kc == KC - 1))
        ost = fs.tile([128, 512], F32, tag="fs", name=f"fo{n}_{m}")
        # alternate copy engines AND DMA rings (SP + ACT both have
        # hardware DGE queues) so nothing single-threads the tail
        if m % 2 == 0:
            nc.vector.tensor_copy(ost[:], ps[:])
            nc.sync.dma_start(out=out[msl, nsl], in_=ost[:])
        else:
            nc.scalar.copy(ost[:], ps[:])
            nc.scalar.dma_start(out=out[msl, nsl], in_=ost[:])

    defer_q = []

    # software-pipelined emission: S runs 2 items ahead of exp/PV
    NIT = len(items)
    emit_S(0)
    emit_exp(0)
    emit_S(1)
    emit_exp(1)
    nc.vector.memset(zall[:, :], 1.0)
    # remaining pair-0 chunks stream inside pair 0 itself; for pairs 1+ the
    # qpT/kpT chunks a pair's first S items touch are emitted a pair early
    # (q chunks first so the lookahead S of the next pair never outruns its
    # DVE producer in queue order)
    # pair-0 chunks stay unsplit: they interleave with vp chunks in the
    # shared ppk pool, and a half-open accumulation there would deadlock
    # the pool's WAR rotation
    kq_work = [(0, c, None) for c in (1, 2, 3, 5)]
    for p in range(1, NPAIR):
        kq_work.append((p, -1, None))
        for c in (4, 5, 0, 1, 2, 3):
            kq_work.append((p, c, 0))
            kq_work.append((p, c, 1))
    kq_i = 0
    for k in range(NIT):
        p_cur, ic_cur, j_cur = items[k]
        if p_cur == 0 and ic_cur == 0:
            emit_vp(j_cur)
        emit_PV_B(k)
        last_of_block = (k == NIT - 1 or items[k + 1][2] == 0)
        if last_of_block:
            # block-end copies must see the completed ozA accumulation,
            # so the A half cannot be deferred past them here
            emit_PV_A(k)
            emit_block_end(k)
            if (p_cur, ic_cur) == (0, 1):
                # vp is done; woT reuses wv_sb's storage for phase 3
                defer_q.append(
                    lambda: nc.sync.dma_start(out=wv_sb[:], in_=woT_r[:]))
            if (p_cur, ic_cur) == (NPAIR - 1, 0):
                # ic=0 normalization + projection half streams into the
                # final block (its inputs are final once (p7, ic0) drains);
                # the reciprocal goes out right away so the DVE computes it
                # under the block's last PVs instead of stalling the stream
                emit_recip()
                for p_ in range(NPAIR):
                    defer_q.append(lambda p=p_: emit_norm(p, 0))
                for n_ in range(2):
                    for m_ in range(4):
                        defer_q.append(lambda n=n_, m=m_: emit_ph3(n, m))
        # interleave upcoming projection work into this pair's stream,
        # half a contraction chunk per slot to keep the PE queue smooth
        slot = ic_cur * NJ + j_cur
        if slot % 2 == 0:
            while kq_i < len(kq_work) and kq_work[kq_i][0] <= p_cur + 1:
                pp_, cc_, hh_ = kq_work[kq_i]
                if cc_ < 0:
                    emit_kq_load(pp_)
                else:
                    emit_kq_chunk(pp_, cc_, hh_)
                kq_i += 1
                if cc_ >= 0:
                    break
        # deferred norm/projection units: one per slot, only while no kq
        # psum accumulation is half-open in the shared ppk pool
        if defer_q and not kq_ps:
            defer_q.pop(0)()
        if k + 2 < NIT:
            emit_S(k + 2)
            emit_exp(k + 2)
        if not last_of_block:
            emit_PV_A(k)
    # drain: ic=1 normalization + remaining output projection
    for fn in defer_q:
        fn()
    emit_recip()
    for p in range(NPAIR):
        emit_norm(p, 1)
    for n in range(2):
        for m in range(4, LQ // 128):
            emit_ph3(n, m)
    attn_ctx.close()


_NC_CACHE = None


def _get_program():
    global _NC_CACHE
    if _NC_CACHE is None:
        _NC_CACHE = build_program()
    return _NC_CACHE


def prep_in_maps(q, k, v, w_q, b_q, w_k, b_k, w_v, b_v, w_o, b_o):
    import ml_dtypes

    f = np.float32
    bf = ml_dtypes.bfloat16
    q, k, v = (np.asarray(t, f) for t in (q, k, v))
    scale = 1.0 / np.sqrt(DH)
    wqT = np.ascontiguousarray((np.asarray(w_q, f) * scale).T).astype(bf)
    wkT = np.ascontiguousarray(np.asarray(w_k, f).T).astype(bf)
    wvT = np.ascontiguousarray(np.asarray(w_v, f).T).astype(bf)
    woT = np.ascontiguousarray(np.asarray(w_o, f).T).astype(bf)
    bqc = np.ascontiguousarray((np.asarray(b_q, f) * scale).reshape(KC, 128).T)
    bkc = np.ascontiguousarray(np.asarray(b_k, f).reshape(KC, 128).T)
    # softmax weights sum to 1, so b_v contributes exactly b_v @ w_o.T to
    # the output; fold it into the output-projection bias
    bor = ((np.asarray(b_v, f) @ np.asarray(w_o, f).T)
           + np.asarray(b_o, f)).reshape(1, D).astype(bf)
    c_or = np.ones((1, 128), bf)
    c_sel = np.zeros((64, 64 * 4 * NPAIR), f)
    for r in range(4 * NPAIR):
        c_sel[r, r * 64:(r + 1) * 64] = 1.0
    in_maps = []
    for c in range(N_CORES):
        b, qh = c // 2, c % 2
        kTb = np.ascontiguousarray(k[b].T).astype(bf)
        vTb = np.ascontiguousarray(v[b].T).astype(bf)
        qTb = np.ascontiguousarray(q[b].T[:, qh * LQ:(qh + 1) * LQ]).astype(bf)
        in_maps.append({
            "xqT": qTb, "xkT": kTb, "xvT": vTb,
            "wqT": wqT, "wkT": wkT, "wvT": wvT, "woT": woT,
            "bqc": bqc, "bkc": bkc, "bor": bor,
            "c_or": c_or, "c_sel": c_sel,
        })
    return in_maps


def run(in_maps, trace=False, **kw):
    nc = _get_program()
    return run_bass_kernel_spmd(nc, in_maps, list(range(N_CORES)),
                                trace=trace, **kw)


def kernel(**inputs):
    in_maps = prep_in_maps(**inputs)
    res = run(in_maps)
    out = np.empty((B, L, D), np.float32)
    for c in range(N_CORES):
        b, qh = c // 2, c % 2
        out[b, qh * LQ:(qh + 1) * LQ, :] = res.results[c]["out"]
    return out


# revision 62
# speedup vs baseline: 1.0313x; 1.0096x over previous
"""Multi-head attention (B=4, L=2048, D=1024, H=16) on 8 Trainium2 NeuronCores.

Sharding: core c = (batch b = c//2, query-half qh = c%2). Each core computes
all 16 heads for its 1024 query rows against the full 2048 keys/values of its
batch. Fully SPMD, no collectives.

All-bf16 data path (fp32 psum). One fully software-pipelined emission
stream: per item (pair, query-half, key-chunk) the S^T matmul pair
(tile_position row-packed, K=64, both heads' scores side by side in one
[128,1024] psum tile) runs two items ahead of the single exp (ACT) and
the PV accumulation, so PE and ACT both stay saturated. Everything else
is interleaved into that stream's slack: the value projection (inside
pair 0's slots, each vpa[j] just before the PV that first needs it),
each next pair's k/q projection chunks (half a contraction chunk per
slot), the ic=0 softmax normalization + first half of the output
projection (inside the final block), with only the ic=1 normalization
and projection half draining at the end. Softmax normalization is
deferred and batched: PV's vpa ones-column yields Z in psum row 64,
Z rows are DMA-gathered into zall, two batched reciprocals + per-pair
selector matmuls broadcast 1/Z, one DVE mul per (pair, ic) normalizes
outU in place. All activations are SBUF-resident end to end (x, w,
outU loaded/kept once; woT reuses wv_sb's storage; no DRAM scratch).
b_v is folded into the output bias host-side (softmax weights sum
to 1). HW exec ~468us vs 1059us for the fp32r phase-serial baseline.
"""

import sys

if "/opt/trn_rl_repo" not in sys.path:
    sys.path.insert(0, "/opt/trn_rl_repo")

import numpy as np

import concourse.bacc as bacc
import concourse.tile as tile
from concourse import mybir
from concourse.bass_utils import run_bass_kernel_spmd

N_CORES = 8
B, L, D = 4, 2048, 1024
NH, DH = 16, 64          # heads, head dim
LQ = L // 2              # query rows per core
F32 = mybir.dt.float32
F32R = mybir.dt.float32r
BF16 = mybir.dt.bfloat16

KC = D // 128            # 8 contraction chunks for projections
NJ = L // 128            # 16 key j-chunks
NI = LQ // 512           # 2 query i-chunks of 512
NPAIR = NH // 2          # 8 head pairs
EXPF = mybir.ActivationFunctionType.Exp


def build_program():
    nc = bacc.Bacc("TRN2", target_bir_lowering=False, debug=False,
                   num_devices=N_CORES)
    with tile.TileContext(nc) as tc:
        _emit(nc, tc)
    nc.compile()
    return nc


def _emit(nc, tc):
    from contextlib import ExitStack

    top = ExitStack()
    dram = top.enter_context(tc.tile_pool(name="dram", bufs=1, space="DRAM"))

    def din(shape, dt, name):
        return dram.tile(shape, dt, kind="ExternalInput", name=name,
                         uniquify=False)

    xqT = din([D, LQ], BF16, "xqT")
    xkT = din([D, L], BF16, "xkT")
    xvT = din([D, L], BF16, "xvT")
    wqT = din([D, D], BF16, "wqT")
    wkT = din([D, D], BF16, "wkT")
    wvT = din([D, D], BF16, "wvT")
    woT = din([D, D], BF16, "woT")
    bqc = din([128, KC], F32, "bqc")
    bkc = din([128, KC], F32, "bkc")
    bor = din([1, D], BF16, "bor")
    c_or = din([1, 128], BF16, "c_or")
    c_sel = din([64, 64 * 4 * NPAIR], F32R, "c_sel")
    out = dram.tile([LQ, D], F32, kind="ExternalOutput", name="out",
                    uniquify=False)

    # ---- persistent SBUF -------------------------------------------------
    pers = top.enter_context(tc.tile_pool(name="pers", bufs=1))
    kpT = [pers.tile([128, L], BF16, name=f"kpT{m}") for m in range(NPAIR)]
    qpT = [pers.tile([128, LQ], BF16, name=f"qpT{m}") for m in range(NPAIR)]
    # vpa: per j-chunk, 16 heads x (64 value cols + 1 ones col + 1 pad col
    # so every head's 65-col weight slice starts 4B-aligned)
    VST = 66
    vpa = [pers.tile([128, NH * VST], BF16, name=f"vpa{m}") for m in range(NJ)]
    outU = [pers.tile([128, LQ], BF16, name=f"outU{m}") for m in range(NPAIR)]
    xk_sb = pers.tile([128, KC, L], BF16, name="xk_sb")
    xq_sb = pers.tile([128, KC, LQ], BF16, name="xq_sb")
    wv_sb = pers.tile([128, KC, D], BF16, name="wv_sb")
    ones1 = pers.tile([1, 128], BF16, name="ones1")
    # b_v's contribution to the output is bv @ woT (softmax weights sum to
    # 1), folded into bor host-side; no bias term needed in vp itself.
    # 64 partitions (rows 32+ pad: zall 1.0, sel 0) for valid 64x64 PE tiling
    sel_sb = pers.tile([64, 64 * 4 * NPAIR], F32R, name="sel_sb")
    zall = pers.tile([64, 512], F32, name="zall")
    bq_sb = pers.tile([128, KC], F32, name="bq_sb")
    bk_sb = pers.tile([128, KC], F32, name="bk_sb")
    bo_sb = pers.tile([1, D], BF16, name="bo_sb")

    xkT_r = xkT.rearrange("(kc p) l -> p kc l", p=128)
    xqT_r = xqT.rearrange("(kc p) l -> p kc l", p=128)
    xvT_r = xvT.rearrange("(kc p) l -> p kc l", p=128)
    wqT_r = wqT.rearrange("(kc p) m -> p kc m", p=128)
    wkT_r = wkT.rearrange("(kc p) m -> p kc m", p=128)
    wvT_r = wvT.rearrange("(kc p) m -> p kc m", p=128)
    woT_r = woT.rearrange("(kc p) m -> p kc m", p=128)

    # ---- phases 1 + 2: projections interleaved with attention -----------
    # PSUM budget (8 banks): psAB 2 bufs x [128,1024] = 4, psO 2 tags x 1
    # buf x [65,512] = 2, ppk (kq chunks AND vp chunks) 2 x [128,512] = 2.
    attn_ctx = ExitStack()
    pw = attn_ctx.enter_context(tc.tile_pool(name="pw", bufs=2))
    pxv = attn_ctx.enter_context(tc.tile_pool(name="pxv", bufs=3))
    ppk = attn_ctx.enter_context(tc.tile_pool(name="ppk", bufs=2, space="PSUM"))
    psA = attn_ctx.enter_context(tc.tile_pool(name="psA", bufs=2, space="PSUM"))
    psO = attn_ctx.enter_context(tc.tile_pool(name="psO", bufs=1, space="PSUM"))
    pe = attn_ctx.enter_context(tc.tile_pool(name="pe", bufs=3))
    prc = attn_ctx.enter_context(tc.tile_pool(name="prc", bufs=2))

    xv_pre = {}

    def stage_xv(m):
        xb = pxv.tile([128, KC, 128], BF16, tag="xv", name=f"xv{m}")
        nc.sync.dma_start(out=xb[:], in_=xvT_r[:, :, m * 128:(m + 1) * 128])
        xv_pre[m] = xb

    def emit_vp(m):
        # value projection for key chunk m -> vpa[m] (emitted just before
        # the PV that first consumes it, inside pair 0's stream)
        if m not in xv_pre:
            stage_xv(m)
        xb = xv_pre.pop(m)
        va = vpa[m].rearrange("p (h c) -> p h c", c=VST)
        nc.vector.memset(va[:, :, 64:65], 1.0)
        for n in range(2):
            nsl = slice(n * 512, (n + 1) * 512)
            ps = ppk.tile([128, 512], F32, tag="pk", name=f"pv{m}_{n}")
            for kc in range(KC):
                nc.tensor.matmul(ps[:], xb[:, kc, :], wv_sb[:, kc, nsl],
                                 start=(kc == 0), stop=(kc == KC - 1))
            nc.vector.tensor_copy(va[:, 8 * n:8 * (n + 1), 0:64], ps[:])

    wk_tiles = {}

    def emit_kq_load(p):
        wkb = pw.tile([128, KC, 128], BF16, tag="wk")
        nc.sync.dma_start(out=wkb[:], in_=wkT_r[:, :, p * 128:(p + 1) * 128])
        wqb = pw.tile([128, KC, 128], BF16, tag="wq")
        nc.sync.dma_start(out=wqb[:], in_=wqT_r[:, :, p * 128:(p + 1) * 128])
        wk_tiles[p] = (wkb, wqb)

    kq_ps = {}

    def emit_kq_chunk(p, c, half=None):
        # chunks 0-3: kpT[p] 512-col chunk c; chunks 4-5: qpT[p] chunk c-4
        # half=0/1 emits only the lower/upper kc contraction half (so the
        # matmul burst can be split across two pipeline slots)
        wkb, wqb = wk_tiles[p]
        if c < 4:
            wb, x_sb, dst, bias, cc = wkb, xk_sb, kpT[p], bk_sb, c
        else:
            wb, x_sb, dst, bias, cc = wqb, xq_sb, qpT[p], bq_sb, c - 4
        csl = slice(cc * 512, (cc + 1) * 512)
        if half in (None, 0):
            ps = ppk.tile([128, 512], F32, tag="pk", name=f"pk{p}_{c}")
            kq_ps[(p, c)] = ps
        else:
            ps = kq_ps.pop((p, c))
        kcs = range(KC) if half is None else range(half * KC // 2,
                                                  (half + 1) * KC // 2)
        for kc in kcs:
            nc.tensor.matmul(ps[:], wb[:, kc, :], x_sb[:, kc, csl],
                             start=(kc == 0), stop=(kc == KC - 1))
        if half in (None, 1):
            nc.vector.tensor_scalar_add(dst[:, csl], ps[:], bias[:, p:p + 1])

    # pair-0 prefix: just the chunks the first S items need (kpT cols 0:512,
    # qpT ic=0); the rest streams inside pair 0's item slots. DMAs are
    # emitted in critical-path order: pair-0 weights + first x slices first
    # so the PE (and then ACT) starts within a few us.
    emit_kq_load(0)
    nc.sync.dma_start(out=xk_sb[:, :, 0:512], in_=xkT_r[:, :, 0:512])
    nc.sync.dma_start(out=xq_sb[:, :, 0:512], in_=xqT_r[:, :, 0:512])
    nc.sync.dma_start(out=bq_sb[:], in_=bqc[:])
    nc.sync.dma_start(out=bk_sb[:], in_=bkc[:])
    emit_kq_chunk(0, 0)
    emit_kq_chunk(0, 4)
    nc.sync.dma_start(out=wv_sb[:], in_=wvT_r[:])
    for m_ in range(3):
        stage_xv(m_)
    nc.sync.dma_start(out=ones1[:], in_=c_or[:])
    for c in range(1, 4):
        nc.sync.dma_start(out=xk_sb[:, :, c * 512:(c + 1) * 512],
                          in_=xkT_r[:, :, c * 512:(c + 1) * 512])
    nc.sync.dma_start(out=xq_sb[:, :, 512:1024], in_=xqT_r[:, :, 512:1024])
    nc.sync.dma_start(out=sel_sb[:], in_=c_sel[:])
    nc.sync.dma_start(out=bo_sb[:], in_=bor[:])

    # flattened item stream: one item = one key j-chunk of one (pair, ic)
    # block. Scores for both heads of the pair sit side by side in one
    # [128, 1024] psum tile so a single exp op covers them.
    items = [(p, ic, j)
             for p in range(NPAIR) for ic in range(NI) for j in range(NJ)]
    s_t, e_t = {}, {}
    oz = {}

    def emit_S(k):
        p, ic, j = items[k]
        isl = slice(ic * 512, (ic + 1) * 512)
        jsl = slice(j * 128, (j + 1) * 128)
        s = psA.tile([128, 1024], F32, tag="s")
        nc.tensor.matmul(s[:, 0:512], kpT[p][0:64, jsl], qpT[p][0:64, isl],
                         tile_position=(0, 0))
        nc.tensor.matmul(s[:, 512:1024], kpT[p][64:128, jsl],
                         qpT[p][64:128, isl], tile_position=(64, 0))
        s_t[k] = s

    def emit_exp(k):
        s = s_t.pop(k)
        e = pe.tile([128, 1024], BF16, tag="e")
        nc.scalar.activation(e[:], s[:], EXPF)
        e_t[k] = e

    def emit_PV(k):
        p, ic, j = items[k]
        hA, hB = 2 * p, 2 * p + 1
        if j == 0:
            ozA = psO.tile([65, 512], F32, tag="oa", name=f"ozA{p}_{ic}")
            ozB = psO.tile([65, 512], F32, tag="ob", name=f"ozB{p}_{ic}")
            oz[(p, ic)] = (ozA, ozB)
        ozA, ozB = oz[(p, ic)]
        e = e_t.pop(k)
        # B half first: the second PV after an S pair consistently ran
        # ~100ns slower; give the B matmul the better prefetch slot
        nc.tensor.matmul(ozB[:, :], vpa[j][:, hB * VST:hB * VST + 65],
                         e[:, 512:1024], start=(j == 0), stop=(j == NJ - 1))
        nc.tensor.matmul(ozA[:, :], vpa[j][:, hA * VST:hA * VST + 65],
                         e[:, 0:512], start=(j == 0), stop=(j == NJ - 1))

    def emit_block_end(k):
        p, ic, j = items[k]
        ozA, ozB = oz.pop((p, ic))
        isl = slice(ic * 512, (ic + 1) * 512)
        # unnormalized head outputs -> SBUF (normalized in place later)
        nc.vector.tensor_copy(outU[p][0:64, isl], ozA[0:64, :])
        nc.vector.tensor_copy(outU[p][64:128, isl], ozB[0:64, :])
        # Z rows (psum partition 64) -> zall rows via staging + DMA
        zst = prc.tile([65, 1024], F32, tag="zs")
        nc.vector.tensor_copy(zst[64:65, 0:512], ozA[64:65, :])
        nc.vector.tensor_copy(zst[64:65, 512:1024], ozB[64:65, :])
        r0 = 4 * p + 2 * ic
        nc.sync.dma_start(out=zall[r0:r0 + 1, :], in_=zst[64:65, 0:512])
        nc.sync.dma_start(out=zall[r0 + 1:r0 + 2, :], in_=zst[64:65, 512:1024])

    # ---- deferred units: per-block normalization + output projection ----
    # Each block's softmax normalization streams into the item loop right
    # after the block drains (a full-table reciprocal is cheap and
    # idempotent on already-final rows); the ic=0 half of the output
    # projection streams into the last block. Only pair 7's ic=1 norm and
    # the ic=1 projection half remain after the loop.
    fs = attn_ctx.enter_context(tc.tile_pool(name="fs", bufs=2))
    pn = attn_ctx.enter_context(tc.tile_pool(name="pn", bufs=1))
    rz = pn.tile([64, 512], F32R, name="rz")

    def emit_recip():
        # one batched reciprocal per ic phase — the DVE reciprocal is an
        # iterative ~4us op, so per-block recips stall the PE badly
        with nc.allow_low_precision(reason="fp32r rounding of 1/Z"):
            nc.vector.reciprocal(rz[:], zall[:])

    def emit_norm(p, ic):
        r0 = 4 * p + 2 * ic
        isl = slice(ic * 512, (ic + 1) * 512)
        rzb = ppk.tile([128, 512], F32, tag="pk", name=f"rzb{p}_{ic}")
        nc.tensor.matmul(rzb[:, :], sel_sb[:, r0 * 64:(r0 + 2) * 64], rz[:])
        nc.vector.tensor_mul(outU[p][:, isl], outU[p][:, isl], rzb[:])

    def emit_ph3(n, m):
        nsl = slice(n * 512, (n + 1) * 512)
        msl = slice(m * 128, (m + 1) * 128)
        ps = ppk.tile([128, 512], F32, tag="pk", name=f"pf{n}_{m}")
        nc.tensor.matmul(ps[:], ones1[0:1, :], bo_sb[0:1, nsl],
                         start=True, stop=False)
        for kc in range(KC):
            nc.tensor.matmul(ps[:], outU[kc][:, msl], wv_sb[:, kc, nsl],
                             start=False, stop=(kc == KC - 1))
        ost = fs.tile([128, 512], F32, tag="fs", name=f"fo{n}_{m}")
        # alternate copy engines AND DMA rings (SP + ACT both have
        # hardware DGE queues) so nothing single-threads the tail
        if m % 2 == 0:
            nc.vector.tensor_copy(ost[:], ps[:])
            nc.sync.dma_start(out=out[msl, nsl], in_=ost[:])
        else:
            nc.scalar.copy(ost[:], ps[:])
            nc.scalar.dma_start(out=out[msl, nsl], in_=ost[:])

    defer_q = []

    # software-pipelined emission: S runs 2 items ahead of exp/PV
    NIT = len(items)
    emit_S(0)
    emit_exp(0)
    emit_S(1)
    emit_exp(1)
    nc.vector.memset(zall[:, :], 1.0)
    # remaining pair-0 chunks stream inside pair 0 itself; for pairs 1+ the
    # qpT/kpT chunks a pair's first S items touch are emitted a pair early
    # (q chunks first so the lookahead S of the next pair never outruns its
    # DVE producer in queue order)
    # pair-0 chunks stay unsplit: they interleave with vp chunks in the
    # shared ppk pool, and a half-open accumulation there would deadlock
    # the pool's WAR rotation
    kq_work = [(0, c, None) for c in (1, 2, 3, 5)]
    for p in range(1, NPAIR):
        kq_work.append((p, -1, None))
        for c in (4, 5, 0, 1, 2, 3):
            kq_work.append((p, c, 0))
            kq_work.append((p, c, 1))
    kq_i = 0
    for k in range(NIT):
        p_cur, ic_cur, j_cur = items[k]
        if p_cur == 0 and ic_cur == 0:
            emit_vp(j_cur)
        emit_PV(k)
        if k == NIT - 1 or items[k + 1][2] == 0:
            emit_block_end(k)
            if (p_cur, ic_cur) == (0, 1):
                # vp is done; woT reuses wv_sb's storage for phase 3
                defer_q.append(
                    lambda: nc.sync.dma_start(out=wv_sb[:], in_=woT_r[:]))
            if (p_cur, ic_cur) == (NPAIR - 1, 0):
                # ic=0 normalization + projection half streams into the
                # final block (its inputs are final once (p7, ic0) drains);
                # the reciprocal goes out right away so the DVE computes it
                # under the block's last PVs instead of stalling the stream
                emit_recip()
                for p_ in range(NPAIR):
                    defer_q.append(lambda p=p_: emit_norm(p, 0))
                for n_ in range(2):
                    for m_ in range(4):
                        defer_q.append(lambda n=n_, m=m_: emit_ph3(n, m))
        # interleave upcoming projection work into this pair's stream,
        # half a contraction chunk per slot to keep the PE queue smooth
        slot = ic_cur * NJ + j_cur
        if slot % 2 == 0:
            while kq_i < len(kq_work) and kq_work[kq_i][0] <= p_cur + 1:
                pp_, cc_, hh_ = kq_work[kq_i]
                if cc_ < 0:
                    emit_kq_load(pp_)
                else:
                    emit_kq_chunk(pp_, cc_, hh_)
                kq_i += 1
                if cc_ >= 0:
                    break
        # deferred norm/projection units: one per slot, only while no kq
        # psum accumulation is half-open in the shared ppk pool
        if defer_q and not kq_ps:
            defer_q.pop(0)()
        if k + 2 < NIT:
            emit_S(k + 2)
            emit_exp(k + 2)
    # drain: ic=1 normalization + remaining output projection
    for fn in defer_q:
        fn()
    emit_recip()
    for p in range(NPAIR):
        emit_norm(p, 1)
    for n in range(2):
        for m in range(4, LQ // 128):
            emit_ph3(n, m)
    attn_ctx.close()


_NC_CACHE = None


def _get_program():
    global _NC_CACHE
    if _NC_CACHE is None:
        _NC_CACHE = build_program()
    return _NC_CACHE


def prep_in_maps(q, k, v, w_q, b_q, w_k, b_k, w_v, b_v, w_o, b_o):
    import ml_dtypes

    f = np.float32
    bf = ml_dtypes.bfloat16
    q, k, v = (np.asarray(t, f) for t in (q, k, v))
    scale = 1.0 / np.sqrt(DH)
    wqT = np.ascontiguousarray((np.asarray(w_q, f) * scale).T).astype(bf)
    wkT = np.ascontiguousarray(np.asarray(w_k, f).T).astype(bf)
    wvT = np.ascontiguousarray(np.asarray(w_v, f).T).astype(bf)
    woT = np.ascontiguousarray(np.asarray(w_o, f).T).astype(bf)
    bqc = np.ascontiguousarray((np.asarray(b_q, f) * scale).reshape(KC, 128).T)
    bkc = np.ascontiguousarray(np.asarray(b_k, f).reshape(KC, 128).T)
    # softmax weights sum to 1, so b_v contributes exactly b_v @ w_o.T to
    # the output; fold it into the output-projection bias
    bor = ((np.asarray(b_v, f) @ np.asarray(w_o, f).T)
           + np.asarray(b_o, f)).reshape(1, D).astype(bf)
    c_or = np.ones((1, 128), bf)
    c_sel = np.zeros((64, 64 * 4 * NPAIR), f)
    for r in range(4 * NPAIR):
        c_sel[r, r * 64:(r + 1) * 64] = 1.0
    in_maps = []
    for c in range(N_CORES):
        b, qh = c // 2, c % 2
        kTb = np.ascontiguousarray(k[b].T).astype(bf)
        vTb = np.ascontiguousarray(v[b].T).astype(bf)
        qTb = np.ascontiguousarray(q[b].T[:, qh * LQ:(qh + 1) * LQ]).astype(bf)
        in_maps.append({
            "xqT": qTb, "xkT": kTb, "xvT": vTb,
            "wqT": wqT, "wkT": wkT, "wvT": wvT, "woT": woT,
            "bqc": bqc, "bkc": bkc, "bor": bor,
            "c_or": c_or, "c_sel": c_sel,
        })
    return in_maps


def run(in_maps, trace=False, **kw):
    nc = _get_program()
    return run_bass_kernel_spmd(nc, in_maps, list(range(N_CORES)),
                                trace=trace, **kw)


def kernel(**inputs):
    in_maps = prep_in_maps(**inputs)
    res = run(in_maps)
    out = np.empty((B, L, D), np.float32)
    for c in range(N_CORES):
        b, qh = c // 2, c % 2
        out[b, qh * LQ:(qh + 1) * LQ, :] = res.results[c]["out"]
    return out
